# revision 33
# baseline (speedup 1.0000x reference)
"""Trainium2 Bass kernel for DeepFusionBlock sparse knn-attention.

Contract: kernel(**inputs) takes FULL numpy inputs (as in reference
setup_inputs()) and returns the FULL [65536, 256] float32 output.

Strategy: data-parallel over points N across 8 NeuronCores. The knn indices
are known at kernel-build time, so the host expands image_features into
(tile, neighbor, point) column order — one column per (point, neighbor)
pair. With that ordering, each 128-column projection matmul
img_chunk @ [Wk | Wv] lands in PSUM with PARTITION INDEX == POINT INDEX,
i.e. the projection output is already in the attention layout. No DRAM
k/v table, no round-trip, no gather.

  per 128-point tile (64 tiles/core):
    img tile [128ch x 2, 2048 cols]; 4 psum tiles [128, 4, 256] (2 PSUM
      banks each, FOUR neighbors per tile) <- 8 matmuls each
    scores: fused custom-DVE MUL_SCAN (running sum of kgk*q) read
      DIRECTLY from PSUM — one op per psum group; per-neighbor dot
      products recovered as differences of scan values at segment
      boundaries (one strided tensor_tensor subtract), + mask bias,
      then exp + fp32 denominator in ONE scalar op (accum_out).
    weighted V: scalar-engine copies V halves h-major (kgv), then two
      half-size custom-DVE MUL_SCANs (kgv * e16 broadcast) INTERLEAVED
      between the next tile's K-scans so the Vector queue always has
      ready work at tile boundaries (97% DVE occupancy); av recovered
      by the custom SUB_SCALE op ((tails - prevs) * rden) in one DVE op.
    PE transpose -> avT_all; output projection batched 2 tiles per
      256-col Wc matmul pair; outputs shipped bf16 (host up-casts).
    Software pipeline: V-side lags 2 tiles (VDELAY=2); PSUM: 3x
      double-buffered kv group tiles + q bank + shared avT/fout bank.

Host un-transposes/assembles the final [65536, 256] f32 output.
"""

import sys

for _p in ("/opt/trn_rl_repo",):
    if _p not in sys.path:
        sys.path.insert(0, _p)

import numpy as np
import ml_dtypes

import concourse.bass as bass
import concourse.bacc as bacc
import concourse.mybir as mybir
import concourse.tile as tile
from concourse import bass_utils

BF16 = ml_dtypes.bfloat16

P = 128          # partitions / tile height
K = 16           # knn neighbors
H = 128          # head dim
CL = 128         # lidar channels
CI = 256         # image channels
N_CORES = 8
EPS = 1e-30
NEG = -1.0e30    # score bias for invalid rows
NG = 4           # neighbors per psum group
NGRP = K // NG   # psum groups per tile

# --- custom DVE ops --------------------------------------------------------
from concourse import dve_ops as _dve_ops
from concourse.dve_spec import Spec, Src0, Src1, C0, AluOp, scan, lower
from concourse.dve_ops import DveOp
from concourse.dve_uop import DveOpSpec


def _computed_sha(spec):
    shas = {}
    for ver in ("v3", "v4"):
        s = DveOpSpec(name="x", opcode=1, uops=lower(spec, ver=ver), rd1_en=True)
        shas[ver] = s.sha(ver)
    return shas


def _register(name, spec):
    if name in _dve_ops._SUB_OPCODE_FOR_NAME:
        return next(op for op in _dve_ops.OPS if op.name == name)
    op = DveOp(name, spec, subdim=False, uops_sha=_computed_sha(spec))
    row = max(_dve_ops._SUB_OPCODE_FOR_NAME.values()) + 1
    assert row < 0x20
    _dve_ops.OPS.append(op)
    _dve_ops._SUB_OPCODE_FOR_NAME[name] = row
    _dve_ops.CUSTOM_DVE_SPECS[name] = op.spec
    return op


def _ref_mul_scan(in0, in1, c0, c1, c2):
    prod = in0.astype(np.float32) * in1.astype(np.float32)
    flat = prod.reshape(prod.shape[0], -1)
    return np.cumsum(flat, axis=1).reshape(prod.shape).astype(np.float32)


def _ref_sub_scale(in0, in1, c0, c1, c2):
    return ((in0.astype(np.float32) - in1.astype(np.float32)) * c0).astype(
        np.float32
    )


MUL_SCAN = _register(
    "MUL_SCAN_ANT",
    Spec(body=scan(AluOp.ADD, Src0 * Src1), reference=_ref_mul_scan),
)
SUB_SCALE = _register(
    "SUB_SCALE_ANT",
    Spec(body=(Src0 - Src1) * C0, reference=_ref_sub_scale),
)


def build_body(tc, outs, ins, n_pts, bias_kv=False, bias_q=False):
    """Trace the device program into TileContext tc.

    ins: dict of DRAM APs:
      imgrT [2, 128, n_refs] bf16   (expanded image rows, transposed;
                                     column (t*16+m)*128+p = image row
                                     knn_ids[t*128+p, m])
      lidarT[128, n_pts]     bf16
      fbT   [128, n_tiles*K] bf16   (0 if ref valid else -1e30)
      wq    [128, 128]  bf16        (Wq / sqrt(H))
      wkv   [2, 128, 256] bf16      (chunk a: [Wk_a | Wv_a])
      wc    [2, 128, 128] bf16      (Wc output-channel halves)
      bc2   [2, 128, 1] f32         (bc output-channel halves)
      (optional) bkv [1, 256] f32, bq2 [1, 128] f32
    outs: dict with outT [2, 128, n_pts] bf16
    """
    nc = tc.nc
    fp32 = mybir.dt.float32
    bf16 = mybir.dt.bfloat16
    OP = mybir.AluOpType
    ACTF = mybir.ActivationFunctionType

    imgrT = ins["imgrT"]
    lidarT = ins["lidarT"]
    outT = outs["outT"]

    n_tiles = n_pts // P

    with tc.tile_pool(name="consts", bufs=1) as cpool:
        wq_sb = cpool.tile([P, H], bf16)
        nc.sync.dma_start(out=wq_sb[:], in_=ins["wq"][:, :])
        wkv_sb = cpool.tile([P, 2, CI], bf16)
        nc.sync.dma_start(out=wkv_sb[:], in_=ins["wkv"].rearrange("a p j -> p a j"))
        wc_sb = cpool.tile([P, 2, H], bf16)
        nc.sync.dma_start(out=wc_sb[:], in_=ins["wc"].rearrange("a p j -> p a j"))
        bc_sb = cpool.tile([P, 2], fp32)
        nc.sync.dma_start(out=bc_sb[:], in_=ins["bc2"].rearrange("a p o -> p (a o)"))
        fb_sb = cpool.tile([P, n_tiles * K], bf16)
        nc.sync.dma_start(out=fb_sb[:], in_=ins["fbT"][:, :])
        ident = cpool.tile([P, P], bf16)
        from concourse.masks import make_identity
        make_identity(nc, ident[:])
        avT_all = cpool.tile([P, n_pts], bf16)
        eps_sb = cpool.tile([P, 1], fp32)
        nc.gpsimd.memset(eps_sb[:], EPS)
        # scan pads: column 0 of each row stays 0 (seed for tail diffs)
        kpads = []
        vpads = []
        for b in range(3):
            kp = cpool.tile([P, NGRP, NG * P + 1], fp32, tag=f"kp{b}")
            nc.gpsimd.memset(kp[:, :, 0:1], 0.0)
            kpads.append(kp)
            vp = cpool.tile([P, H * K + 2], fp32, tag=f"vp{b}")
            nc.gpsimd.memset(vp[:, 0:1], 0.0)
            nc.gpsimd.memset(vp[:, H * K // 2 + 1 : H * K // 2 + 2], 0.0)
            vpads.append(vp)
        if bias_kv:
            bkv_sb = cpool.tile([1, CI], fp32)
            nc.sync.dma_start(out=bkv_sb[:], in_=ins["bkv"][:, :])
        if bias_q:
            bq_sb = cpool.tile([1, H], fp32)
            nc.sync.dma_start(out=bq_sb[:], in_=ins["bq2"][:, :])
        if bias_kv or bias_q:
            ones1 = cpool.tile([1, P], bf16)
            nc.gpsimd.memset(ones1[:], 1.0)

        with (
            tc.tile_pool(name="p_img", bufs=6) as pimg,
            tc.tile_pool(name="p_kg", bufs=5) as pkg,
            tc.tile_pool(name="p_med", bufs=6) as pb,
            tc.tile_pool(name="p_small", bufs=10) as pbs,
            tc.tile_pool(name="ps_kv", bufs=3, space="PSUM") as pkv,
            tc.tile_pool(name="ps_q", bufs=1, space="PSUM") as pq,
            tc.tile_pool(name="ps_o", bufs=1, space="PSUM") as po,
        ):
            pending = []
            FW = 2 * P

            def emit_fout_chunk(c):
                # batched output projection for 4 tiles: two 512-col
                # matmuls replace eight 128-col ones
                for a in range(2):
                    f_ps = po.tile([P, FW], fp32, tag="fch")  # shares slot with avT_ps
                    nc.tensor.matmul(
                        f_ps[:], lhsT=wc_sb[:, a, :],
                        rhs=avT_all[:, c * FW : (c + 1) * FW],
                        start=True, stop=True,
                    )
                    fo = pb.tile([P, FW], bf16, tag="fo")
                    nc.scalar.activation(
                        out=fo[:], in_=f_ps[:], func=ACTF.Identity,
                        bias=bc_sb[:, a : a + 1], scale=1.0,
                    )
                    nc.sync.dma_start(
                        out=outT[a, :, c * FW : (c + 1) * FW], in_=fo[:]
                    )

            HB = H * K // 2   # elements per half V-scan

            def emit_v_p1(t0, kgv, e16, den2, vpad):
                # V weighted sum, first half: recip + fused mul-scan h<64
                rden = pbs.tile([P, 1], fp32, tag="rden")
                nc.vector.reciprocal(out=rden[:], in_=den2[:])
                e_b = e16[:, :]
                e_bcast = bass.AP(
                    e_b.tensor, e_b.offset, [e_b.ap[0], [0, H // 2], e_b.ap[1]]
                )
                vout = vpad[:, 1 : HB + 1].rearrange(
                    "p (h m) -> p h m", h=H // 2
                )
                nc.vector._custom_dve(MUL_SCAN, out=vout,
                                      in0=kgv[:, 0 : H // 2, :], in1=e_bcast)
                return rden

            def emit_v_p2(t0, kgv, e16, den2, vpad):
                # second half-scan h>=64 into the region after the zero col
                e_b = e16[:, :]
                e_bcast = bass.AP(
                    e_b.tensor, e_b.offset, [e_b.ap[0], [0, H // 2], e_b.ap[1]]
                )
                vout = vpad[:, HB + 2 : 2 * HB + 2].rearrange(
                    "p (h m) -> p h m", h=H // 2
                )
                nc.vector._custom_dve(MUL_SCAN, out=vout,
                                      in0=kgv[:, H // 2 : H, :], in1=e_bcast)

            def emit_v_p3(t0, rden, vpad):
                # av = (tails - prevs) * rden across both halves, transpose
                ap0 = vpad[:, 0:1]
                blk = HB + 1
                tails = bass.AP(
                    ap0.tensor, ap0.offset + K,
                    [ap0.ap[0], [blk, 2], [K, H // 2]],
                )
                prevs = bass.AP(
                    ap0.tensor, ap0.offset,
                    [ap0.ap[0], [blk, 2], [K, H // 2]],
                )
                avb = pb.tile([P, H], bf16, tag="avb")
                nc.vector._custom_dve(
                    SUB_SCALE,
                    out=avb[:].rearrange("p (a b) -> p a b", a=2),
                    in0=tails, in1=prevs, s0=rden[:, 0:1],
                )
                avT_ps = po.tile([P, P], bf16, tag="fch")
                nc.tensor.transpose(avT_ps[:], avb[:], ident[:])
                nc.scalar.copy(out=avT_all[:, t0 : t0 + P], in_=avT_ps[:])

            VDELAY = 2

            def emit_vparts(args, upto, state):
                # emit V-side pieces 1..upto for the pending tile
                t0, kgv, e16, den2, vpad = args
                if state["done"] < 1 <= upto:
                    state["rden"] = emit_v_p1(t0, kgv, e16, den2, vpad)
                    state["done"] = 1
                if state["done"] < 2 <= upto:
                    emit_v_p2(t0, kgv, e16, den2, vpad)
                    state["done"] = 2
                if state["done"] < 3 <= upto:
                    emit_v_p3(t0, state["rden"], vpad)
                    state["done"] = 3

            vstate = {"done": 3, "rden": None}
            vargs = None
            for t in range(n_tiles):
                if len(pending) >= VDELAY:
                    if vargs is not None:
                        emit_vparts(vargs, 3, vstate)
                    vargs = pending.pop(0)
                    vstate = {"done": 0, "rden": None}
                    emit_vparts(vargs, 1, vstate)
                t0 = t * P
                kpad = kpads[t % 3]
                img = pimg.tile([P, 2, K * P], bf16, tag="img")
                GW = NG * P
                for j in range(NGRP):
                    nc.sync.dma_start(
                        out=img[:, :, j * GW : (j + 1) * GW],
                        in_=imgrT[
                            :, :, t * K * P + j * GW : t * K * P + (j + 1) * GW
                        ].rearrange("a p n -> p a n"),
                    )
                lidc = pb.tile([P, CL], bf16, tag="lidc")
                nc.sync.dma_start(out=lidc[:], in_=lidarT[:, t0 : t0 + P])

                # q projection
                q_ps = pq.tile([P, H], fp32, tag="q_ps")
                nc.tensor.matmul(
                    q_ps[:], lhsT=lidc[:], rhs=wq_sb[:], start=True,
                    stop=not bias_q,
                )
                if bias_q:
                    nc.tensor.matmul(
                        q_ps[:], lhsT=ones1[0:1, :], rhs=bq_sb[0:1, :],
                        start=False, stop=True,
                    )
                q_sb = pb.tile([P, H], bf16, tag="q_sb")
                nc.scalar.copy(out=q_sb[:], in_=q_ps[:])
                q_b = q_sb[:, :]
                q_bcast = bass.AP(
                    q_b.tensor, q_b.offset, [q_b.ap[0], [0, NG], q_b.ap[1]]
                )

                # k/v projection straight into attention layout; FOUR
                # neighbors per two-bank psum tile.
                kgv = pkg.tile([P, H, K], bf16, tag="kgv")

                def emit_kscan(j, ps):
                    # scores: fused mul-scan straight from PSUM K pages
                    kout = kpad[:, j, 1 : NG * P + 1].rearrange(
                        "p (m h) -> p m h", m=NG
                    )
                    nc.vector._custom_dve(
                        MUL_SCAN, out=kout, in0=ps[:, :, 0:H], in1=q_bcast
                    )
                    # V half -> SBUF h-major (scalar engine)
                    nc.scalar.copy(
                        out=kgv[:, :, NG * j : NG * j + NG],
                        in_=ps[:, :, H:CI].rearrange("p a h -> p h a"),
                    )

                kq = []
                for j in range(NGRP):
                    ps = pkv.tile([P, NG, CI], fp32, tag="ps")
                    halves = [0, 1] if not bias_kv else [0, 1, 2]
                    # sequential accumulation pairs: interleaving start
                    # groups across banks corrupts results on HW
                    for pair in ((0,), (1,), (2,), (3,)):
                        for a in halves:
                            for jj in pair:
                                m = NG * j + jj
                                if a < 2:
                                    nc.tensor.matmul(
                                        ps[:, jj, :],
                                        lhsT=img[:, a, m * P : (m + 1) * P],
                                        rhs=wkv_sb[:, a, :],
                                        start=(a == 0),
                                        stop=(a == 1 and not bias_kv),
                                    )
                                else:
                                    nc.tensor.matmul(
                                        ps[:, jj, :], lhsT=ones1[0:1, :],
                                        rhs=bkv_sb[0:1, :], start=False,
                                        stop=True,
                                    )
                    # scans trail the matmuls by one group so the DVE
                    # never catches the PE mid-tile
                    kq.append((j, ps))
                    if len(kq) >= 2:
                        emit_kscan(*kq.pop(0))
                        if vargs is not None:
                            emit_vparts(vargs, j, vstate)
                while kq:
                    emit_kscan(*kq.pop(0))
                if vargs is not None:
                    emit_vparts(vargs, 3, vstate)
                if t >= 3 and (t - 3) % 2 == 0:
                    emit_fout_chunk((t - 3) // 2)

                # score extraction: s16[m] = scan tail diff + mask bias
                ap0 = kpad[:, 0, 0:1]
                row = NG * P + 1
                tails = bass.AP(
                    ap0.tensor, ap0.offset + P,
                    [ap0.ap[0], [row, NGRP], [P, NG]],
                )
                prevs = bass.AP(
                    ap0.tensor, ap0.offset,
                    [ap0.ap[0], [row, NGRP], [P, NG]],
                )
                s16 = pbs.tile([P, K], fp32, tag="s16")
                nc.gpsimd.tensor_tensor(
                    out=s16[:].rearrange("p (a b) -> p a b", a=NGRP),
                    in0=tails, in1=prevs, op=OP.subtract,
                )
                s16f = pbs.tile([P, K], fp32, tag="s16f")
                nc.gpsimd.tensor_tensor(
                    out=s16f[:], in0=s16[:],
                    in1=fb_sb[:, t * K : (t + 1) * K], op=OP.add,
                )
                # exp (bf16) + f32 denominator in one scalar-engine op
                e16 = pbs.tile([P, K], bf16, tag="e16")
                den = pbs.tile([P, 1], fp32, tag="den")
                nc.scalar.activation(out=e16[:], in_=s16f[:], func=ACTF.Exp,
                                     accum_out=den[:])
                den2 = pbs.tile([P, 1], fp32, tag="den2")
                nc.scalar.activation(out=den2[:], in_=den[:],
                                     func=ACTF.Identity,
                                     bias=eps_sb[:, 0:1], scale=1.0)

                # V side is emitted one iteration later so the DVE queue
                # isn't head-of-line blocked on exp/rden
                pending.append((t0, kgv, e16, den2, vpads[t % 3]))
            if vargs is not None:
                emit_vparts(vargs, 3, vstate)
                vargs = None
            while pending:
                args = pending.pop(0)
                st = {"done": 0, "rden": None}
                emit_vparts(args, 3, st)
            for c in range((n_tiles - 3) // 2 + 1, n_tiles * P // FW):
                emit_fout_chunk(c)


def prep_inputs(lidar, image, Wq, bq, Wk, bk, Wv, bv, Wc, bc, knn_ids,
                n_pts_core, n_cores):
    """Host-side: shard + expand image rows by knn + transpose + cast."""
    wq = (Wq.astype(np.float32) / np.sqrt(np.float32(H))).astype(BF16)
    wkv = np.zeros((2, 128, CI), dtype=BF16)
    for a in range(2):
        wkv[a, :, 0:H] = Wk[a * 128 : (a + 1) * 128, :].astype(BF16)
        wkv[a, :, H : 2 * H] = Wv[a * 128 : (a + 1) * 128, :].astype(BF16)
    wc = np.zeros((2, 128, 128), dtype=BF16)
    for a in range(2):
        wc[a] = Wc[:, a * 128 : (a + 1) * 128].astype(BF16)
    bc2 = np.ascontiguousarray(bc.astype(np.float32).reshape(2, 128, 1))
    bias_kv = bool(np.any(bk != 0) or np.any(bv != 0))
    bias_q = bool(np.any(bq != 0))
    common = {"wq": wq, "wkv": wkv, "wc": wc, "bc2": bc2}
    if bias_kv:
        bkv = np.zeros((1, CI), dtype=np.float32)
        bkv[0, 0:H] = bk
        bkv[0, H : 2 * H] = bv
        common["bkv"] = bkv
    if bias_q:
        common["bq2"] = (bq.astype(np.float32) / np.sqrt(np.float32(H))).reshape(
            1, 128
        )
    img_bf = image.astype(BF16)
    fb_row = np.where(
        image.astype(np.float32).sum(axis=1) == 0.0, np.float32(NEG),
        np.float32(0.0),
    ).astype(BF16)                                  # [N]
    n_tiles = n_pts_core // P
    per_core = []
    for c in range(n_cores):
        sl = slice(c * n_pts_core, (c + 1) * n_pts_core)
        lidarT = np.ascontiguousarray(lidar[sl].astype(np.float32).T).astype(BF16)
        # column (t*16+m)*128+p  <->  image row knn_ids[t*128+p, m]
        ids3 = knn_ids[sl].reshape(n_tiles, P, K).transpose(0, 2, 1)
        order = ids3.reshape(-1)
        img_ref = img_bf[order]                     # [n_refs, 256] bf16
        imgrT = np.ascontiguousarray(img_ref.T).reshape(2, 128, -1)
        fbT = np.ascontiguousarray(
            fb_row[ids3].transpose(2, 0, 1).reshape(P, n_tiles * K)
        )
        per_core.append({"lidarT": lidarT, "imgrT": imgrT, "fbT": fbT})
    return common, per_core, bias_kv, bias_q


def build_program(n_pts, shapes, bias_kv=False, bias_q=False, n_cores=N_CORES):
    nc = bacc.Bacc(
        "TRN2",
        target_bir_lowering=False,
        debug=False,
        enable_asserts=False,
        num_devices=n_cores,
    )
    ins = {}
    for name, (shape, dtype) in shapes.items():
        ins[name] = nc.dram_tensor(
            name, list(shape), mybir.dt.from_np(np.dtype(dtype)),
            kind="ExternalInput"
        ).ap()
    outT = nc.dram_tensor(
        "outT", [2, 128, n_pts], mybir.dt.bfloat16, kind="ExternalOutput"
    ).ap()
    with tile.TileContext(nc) as tc:
        build_body(tc, {"outT": outT}, ins, n_pts,
                   bias_kv=bias_kv, bias_q=bias_q)
    nc.compile()
    return nc


def kernel(**inputs):
    lidar = np.asarray(inputs["lidar_features"])
    image = np.asarray(inputs["image_features"])
    knn_ids = np.asarray(inputs["knn_ids"])
    n_total = lidar.shape[0]
    n_pts = n_total // N_CORES

    common, per_core, bias_kv, bias_q = prep_inputs(
        lidar, image, inputs["Wq"], inputs["bq"], inputs["Wk"], inputs["bk"],
        inputs["Wv"], inputs["bv"], inputs["Wc"], inputs["bc"], knn_ids,
        n_pts, N_CORES,
    )
    in_maps = []
    for c in range(N_CORES):
        m = dict(common)
        m.update(per_core[c])
        in_maps.append(m)
    shapes = {k: (v.shape, v.dtype) for k, v in in_maps[0].items()}

    nc = build_program(n_pts, shapes, bias_kv=bias_kv, bias_q=bias_q)
    res = bass_utils.run_bass_kernel_spmd(
        nc, in_maps, core_ids=list(range(N_CORES))
    )
    out = np.empty((n_total, CI), dtype=np.float32)
    for c in range(N_CORES):
        oT = np.asarray(res.results[c]["outT"]).astype(np.float32)
        out[c * n_pts : (c + 1) * n_pts, :] = (
            oT.transpose(2, 0, 1).reshape(n_pts, CI)
        )
    return out


if __name__ == "__main__":
    np.random.seed(0)
    n_pts = 256
    shapes = {
        "imgrT": ((2, 128, n_pts * K), BF16),
        "lidarT": ((128, n_pts), BF16),
        "fbT": ((128, (n_pts // P) * K), BF16),
        "wq": ((128, 128), BF16),
        "wkv": ((2, 128, 256), BF16),
        "wc": ((2, 128, 128), BF16),
        "bc2": ((2, 128, 1), np.float32),
    }
    nc = build_program(n_pts, shapes, n_cores=8)
    print("build OK")
